# revision 2
# baseline (speedup 1.0000x reference)
"""Trainium2 Bass kernel v2 for CustomAttention — windowed block-sparse.

Device token layout per image (582 cols): 6 windows of 97 tokens =
[96 patches (4 grid rows, row-major), CLS copy]. The 3x3-window mask
means a query row r only attends key rows r-1..r+1, so keys for the
query block of rows 4w-1..4w+4 are fully inside window w (plus CLS).

Per block w (w=0..5):
  keys   = window w tokens (97 rows incl CLS copy)       [partition dim]
  queries= device cols [max(0,97w-25), min(582,97w+121)) [free dim]
  scores = k_w.T @ q  -> exp -> * mask_w (precomputed, encodes 3x3
           window + CLS-key primary-window rule + CLS-copy query cols)
  av     = v_w(+ones col).T @ es accumulated in PSUM across blocks
           (overlapping query col ranges chain the accumulation order)
CLS query rides as 6 per-window copies whose partial outputs+denoms are
summed into col 96. Normalization is deferred (denne row 65 -> DRAM ->
reciprocal -> stride-0 broadcast -> multiply), as in v1.

Score/av operands are bf16 (fp32r is 4x slower below 256 free dim);
qkv/proj GEMMs stay fp32r at free dim >= 256. proj weights bf16 to
match bf16 oc tiles.

Sharding: data-parallel over batch across 8 cores (4 images/core).
"""

import numpy as np

import concourse.bass as bass
import concourse.mybir as mybir
from concourse import bacc
from concourse.bass_utils import run_bass_kernel_spmd
from concourse.tile import TileContext

B, N, C = 32, 577, 768
H, D = 12, 64
NCORES = 8
BPC = B // NCORES
NW = 6                      # windows per image
WTOK = 97                   # tokens per window (96 patches + CLS copy)
TV = NW * WTOK              # 582 device tokens per image
T = BPC * TV                # device tokens per core
SCALE = float(D) ** -0.5
F32 = mybir.dt.float32
F32R = mybir.dt.float32r
BF16 = mybir.dt.bfloat16
P = 128

CT = C // P                 # 6 contraction tiles over channels
VCH = [(0, 512), (512, 256)]   # v / proj output feature chunks
QCH = [(0, 512), (512, 70)]    # q/k token chunks (psum bank limit)
TTILES = [(0, 128), (128, 128), (256, 128), (384, 128), (512, 70)]  # proj
AF = mybir.ActivationFunctionType
ALU = mybir.AluOpType
AXL = mybir.AxisListType

# query col range per block
QR = [(max(0, 97 * w - 25), min(TV, 97 * w + 121)) for w in range(NW)]
# es col offset of block w inside its triple tile
ESOFF = {0: 0, 1: 121, 2: 267, 3: 0, 4: 146, 5: 292}
TRIW = [413, 414]           # used width of triple tiles t0/t1
# av accumulators: triple 0 covers query cols [0,315), triple 1 [266,582);
# overlap [266,315) is summed during the stage copy
AV0W, AV1B, AV1W = 315, 266, TV - 266


def _build_masks():
    """mask triples [97, 414] x2, bf16-ready fp32."""
    mt = [np.zeros((WTOK, 414), np.float32) for _ in range(2)]
    for w in range(NW):
        q0, q1 = QR[w]
        m = np.zeros((WTOK, q1 - q0), np.float32)
        for j in range(q1 - q0):
            dc = q0 + j
            v, i = dc // 97, dc % 97
            if i == 96:
                if v == w:
                    m[:96, j] = 1.0
                    m[96, j] = 1.0 if w == 0 else 0.0
            else:
                p = 96 * v + i
                pr, pc = p // 24, p % 24
                kr = 4 * w + np.arange(96) // 24
                kc = np.arange(96) % 24
                m[:96, j] = (np.abs(kr - pr) <= 1) & (np.abs(kc - pc) <= 1)
                m[96, j] = 1.0 if (pr // 4) == w else 0.0
        mt[w // 3][:, ESOFF[w] : ESOFF[w] + (q1 - q0)] = m
    return mt


def _bcast_ap(ap1d, parts):
    """1-row AP -> [parts, n] with partition stride 0 (DRAM-source DMA)."""
    return bass.AP(
        tensor=ap1d.tensor, offset=ap1d.offset, ap=[[0, parts]] + list(ap1d.ap)[-1:]
    )


def _cls_cols(ap2d):
    """[P, 582] AP -> [P, 5, 1] view of CLS-copy cols 193,290,387,484,581."""
    return ap2d.rearrange("p (w t) -> p w t", t=WTOK)[:, 1:NW, 96:97]


def _build_program(dbg=False):
    nc = bacc.Bacc("TRN2", target_bir_lowering=False, debug=False)
    dbg_t = {}
    if dbg:
        for name, shape, dt in [
            ("dbg_q", [P, TV], BF16),
            ("dbg_k", [P, TV], BF16),
            ("dbg_v", [WTOK, H * 65], BF16),
            ("dbg_es", [WTOK, 414], BF16),
            ("dbg_av", [65, TV], BF16),
        ] + [(f"dbg_oc{i}", [P, TV], BF16) for i in range(H // 2)]:
            dbg_t[name] = nc.dram_tensor(name, shape, dt, kind="ExternalOutput").ap()
    xT = nc.dram_tensor("xT", [C, T], BF16, kind="ExternalInput").ap()
    wqkT = nc.dram_tensor("wqkT", [C, 2 * C], BF16, kind="ExternalInput").ap()
    wvT = nc.dram_tensor("wvT", [C, C], BF16, kind="ExternalInput").ap()
    wpT = nc.dram_tensor("wpT", [C, C], BF16, kind="ExternalInput").ap()
    bqk = nc.dram_tensor("bqk", [2 * C], F32, kind="ExternalInput").ap()
    bv = nc.dram_tensor("bv", [C], F32, kind="ExternalInput").ap()
    bp = nc.dram_tensor("bp", [C], F32, kind="ExternalInput").ap()
    maskd = nc.dram_tensor("maskd", [2, WTOK, 414], BF16, kind="ExternalInput").ap()
    ones12 = nc.dram_tensor("ones12", [H], BF16, kind="ExternalInput").ap()
    y = nc.dram_tensor("y", [T, C], F32, kind="ExternalOutput").ap()

    with TileContext(nc) as tc:
        with (
            tc.tile_pool(name="singles", bufs=1) as singles,
            tc.tile_pool(name="xp", bufs=2) as xp,
            tc.tile_pool(name="vtp", bufs=2) as vtp,
            tc.tile_pool(name="qkp", bufs=2) as qkp,
            tc.tile_pool(name="esp", bufs=6) as esp,
            tc.tile_pool(name="stp", bufs=4) as stp,
            tc.tile_pool(name="ocp", bufs=2) as ocp,
            tc.tile_pool(name="rcp", bufs=3) as rcpp,
            tc.tile_pool(name="ysp", bufs=2) as ysp,
            tc.tile_pool(name="pmm", bufs=2, space="PSUM") as pmm,
            tc.tile_pool(name="psc", bufs=2, space="PSUM") as psc,
            tc.tile_pool(name="pav", bufs=2, space="PSUM") as pav,
            tc.tile_pool(name="drp", bufs=2, space="DRAM") as drp,
        ):
            # ---- persistent loads ----
            wqk_sb, wv_sb, wp_sb = [], [], []
            for ct in range(CT):
                t = singles.tile([P, 2 * C], BF16, tag=f"wqk{ct}")
                nc.sync.dma_start(t[:], wqkT[ct * P : (ct + 1) * P, :])
                wqk_sb.append(t)
                t = singles.tile([P, C], BF16, tag=f"wv{ct}")
                nc.sync.dma_start(t[:], wvT[ct * P : (ct + 1) * P, :])
                wv_sb.append(t)
                t = singles.tile([P, C], BF16, tag=f"wp{ct}")
                nc.sync.dma_start(t[:], wpT[ct * P : (ct + 1) * P, :])
                wp_sb.append(t)
            bqk_sb = singles.tile([P, 2 * C // P], F32, tag="bqk")
            nc.sync.dma_start(bqk_sb[:], bqk.rearrange("(o p) -> p o", p=P))
            bv_sb = singles.tile([P, C], F32, tag="bv")
            nc.sync.dma_start(bv_sb[:], _bcast_ap(bv, P))
            bp_sb = singles.tile([P, C], F32, tag="bp")
            nc.sync.dma_start(bp_sb[:], _bcast_ap(bp, P))
            ones_sb = singles.tile([WTOK, H], BF16, tag="ones_sb")
            nc.sync.dma_start(ones_sb[:], _bcast_ap(ones12, WTOK))
            mask_sb = []
            for t in range(2):
                m = singles.tile([WTOK, 414], BF16, tag=f"mask{t}")
                nc.sync.dma_start(m[:], maskd[t])
                mask_sb.append(m)

            def emit_xv(b):
                xT_b = []
                for ct in range(CT):
                    t = xp.tile([P, TV], BF16, tag=f"x{ct}", name=f"x{ct}")
                    nc.sync.dma_start(
                        t[:], xT[ct * P : (ct + 1) * P, b * TV : (b + 1) * TV]
                    )
                    xT_b.append(t)
                v_win = []
                for w in range(NW):
                    vt = vtp.tile([WTOK, H, 65], BF16, tag=f"vt{w}", name=f"vt{w}")
                    nc.vector.tensor_copy(vt[:, :, 64:65], ones_sb[:, :, None])
                    for ci, (c0, csz) in enumerate(VCH):
                        ps = pmm.tile([P, 512], F32, tag="pb", name="ps")
                        for ct in range(CT):
                            nc.tensor.matmul(
                                ps[:WTOK, :csz],
                                xT_b[ct][:, w * WTOK : (w + 1) * WTOK],
                                wv_sb[ct][:, c0 : c0 + csz],
                                start=(ct == 0),
                                stop=(ct == CT - 1),
                            )
                        nh = csz // D
                        h0 = c0 // D
                        nc.vector.tensor_tensor(
                            vt[:WTOK, h0 : h0 + nh, 0:D],
                            ps[:WTOK, :csz].rearrange("p (h d) -> p h d", d=D),
                            bv_sb[:WTOK, c0 : c0 + csz].rearrange(
                                "p (h d) -> p h d", d=D
                            ),
                            ALU.add,
                        )
                    v_win.append(vt)
                if dbg and b == 0:
                    nc.sync.dma_start(
                        dbg_t["dbg_v"],
                        v_win[0][:].rearrange("p h d -> p (h d)"),
                    )
                return xT_b, v_win

            def emit_proj(b, oc_pair):
                for t0, tsz in TTILES:
                    ysb = ysp.tile([P, C], F32, tag="ysb", name="ysb")
                    for ci, (c0, csz) in enumerate(VCH):
                        ps = pmm.tile([P, 512], F32, tag="pb", name="ps")
                        for kp in range(CT):
                            nc.tensor.matmul(
                                ps[:tsz, :csz],
                                oc_pair[kp][:, t0 : t0 + tsz],
                                wp_sb[kp][:, c0 : c0 + csz],
                                start=(kp == 0),
                                stop=(kp == CT - 1),
                            )
                        nc.vector.tensor_tensor(
                            ysb[:tsz, c0 : c0 + csz],
                            ps[:tsz, :csz],
                            bp_sb[:tsz, c0 : c0 + csz],
                            ALU.add,
                        )
                    nc.sync.dma_start(
                        y[b * TV + t0 : b * TV + t0 + tsz, :], ysb[:tsz, :]
                    )

            xT_b, v_win = emit_xv(0)
            for b in range(BPC):
                oc_pair = [
                    ocp.tile([P, TV], BF16, tag=f"oc{hp}", name=f"oc{hp}")
                    for hp in range(H // 2)
                ]
                srd = drp.tile([H, TV], BF16, tag="srd")
                for hp in range(H // 2):
                    # ---- q/k generation (feature-major, bf16) ----
                    qt = qkp.tile([P, TV], BF16, tag="qk_q")
                    kt = qkp.tile([P, TV], BF16, tag="qk_k")
                    for dst, ft in ((qt, hp), (kt, CT + hp)):
                        for c0, csz in QCH:
                            ps = pmm.tile([P, 512], F32, tag="pb", name="ps")
                            for ct in range(CT):
                                nc.tensor.matmul(
                                    ps[:, :csz],
                                    wqk_sb[ct][:, ft * P : (ft + 1) * P],
                                    xT_b[ct][:, c0 : c0 + csz],
                                    start=(ct == 0),
                                    stop=(ct == CT - 1),
                                )
                            nc.vector.tensor_tensor(
                                dst[:, c0 : c0 + csz],
                                ps[:, :csz],
                                bqk_sb[:, ft : ft + 1].to_broadcast([P, csz]),
                                ALU.add,
                            )
                    if dbg and b == 0 and hp == 0:
                        nc.sync.dma_start(dbg_t["dbg_q"], qt[:])
                        nc.sync.dma_start(dbg_t["dbg_k"], kt[:])

                    # ---- scores + exp + mask per (hi, triple) ----
                    es_t = {0: [], 1: []}
                    for hi in range(2):
                        po = 64 * hi
                        for tri in range(2):
                            sc = psc.tile([WTOK, 414], F32, tag="sc", name="sc")
                            for w in range(tri * 3, tri * 3 + 3):
                                q0, q1 = QR[w]
                                off = ESOFF[w]
                                nc.tensor.matmul(
                                    sc[:WTOK, off : off + (q1 - q0)],
                                    kt[po : po + D, w * WTOK : (w + 1) * WTOK],
                                    qt[po : po + D, q0:q1],
                                    start=True,
                                    stop=True,
                                )
                            es = esp.tile([WTOK, 414], BF16, tag="es", name="es")
                            nc.scalar.activation(
                                es[:, : TRIW[tri]], sc[:, : TRIW[tri]], AF.Exp
                            )
                            nc.vector.tensor_tensor(
                                es[:, : TRIW[tri]],
                                es[:, : TRIW[tri]],
                                mask_sb[tri][:, : TRIW[tri]],
                                ALU.mult,
                            )
                            es_t[hi].append(es)
                            if dbg and b == 0 and hp == 0 and hi == 0 and tri == 0:
                                nc.sync.dma_start(dbg_t["dbg_es"], es[:])

                    # ---- av accumulation + stage + denom/oc routing ----
                    for hi in range(2):
                        h = 2 * hp + hi
                        avp = pav.tile([65, TV], F32, tag="avp", name="avp")
                        # start=True clears the whole bank's has_written bits,
                        # so the overlapping-slice accumulation chain must
                        # execute exactly in w order — force it atomic.
                        with tc.tile_critical():
                            for w in range(NW):
                                q0, q1 = QR[w]
                                es = es_t[hi][w // 3]
                                off = ESOFF[w]
                                lhs = v_win[w][:WTOK, h, :]
                                if w < 5:
                                    nc.tensor.matmul(
                                        avp[0:65, q0:q1],
                                        lhs,
                                        es[:WTOK, off : off + (q1 - q0)],
                                        start=(w == 0),
                                        stop=False,
                                        skip_group_check=True,
                                    )
                                else:
                                    nc.tensor.matmul(
                                        avp[0:65, q0:512],
                                        lhs,
                                        es[:WTOK, off : off + (512 - q0)],
                                        start=False,
                                        stop=True,
                                        skip_group_check=True,
                                    )
                                    nc.tensor.matmul(
                                        avp[0:65, 512:TV],
                                        lhs,
                                        es[:WTOK, off + (512 - q0) : off + (q1 - q0)],
                                        start=True,
                                        stop=True,
                                        skip_group_check=True,
                                    )
                        stg = stp.tile([65, TV], BF16, tag="stg", name="stg")
                        nc.vector.tensor_copy(stg[:], avp[:])
                        if dbg and b == 0 and h == 0:
                            nc.sync.dma_start(dbg_t["dbg_av"], stg[:65, :])
                        nc.sync.dma_start(srd[h : h + 1, :], stg[64:65, :])
                        nc.sync.dma_start(
                            oc_pair[hp][64 * hi : 64 * hi + 64, :], stg[0:64, :]
                        )

                    # ---- CLS partial-output summation into col 96 ----
                    oc = oc_pair[hp]
                    clst = rcpp.tile([P, 1], F32, tag="clst")
                    nc.vector.reduce_sum(
                        clst[:, 0:1], _cls_cols(oc[:, :]), axis=AXL.XY
                    )
                    nc.vector.tensor_tensor(
                        oc[:, 96:97], oc[:, 96:97], clst[:, 0:1], ALU.add
                    )

                    # ---- batched normalization per 3 pairs ----
                    if hp in (2, 5):
                        g0 = 6 * (hp // 3)
                        srs = rcpp.tile([6, TV], BF16, tag="srs")
                        nc.sync.dma_start(srs[:], srd[g0 : g0 + 6, :])
                        clsd = rcpp.tile([6, 1], F32, tag="clsd")
                        nc.vector.reduce_sum(
                            clsd[:, 0:1], _cls_cols(srs[:, :]), axis=AXL.XY
                        )
                        nc.vector.tensor_tensor(
                            srs[:, 96:97], srs[:, 96:97], clsd[:, 0:1], ALU.add
                        )
                        rr = rcpp.tile([6, TV], F32, tag="rr")
                        nc.vector.reciprocal(rr[:], srs[:])
                        rrd = drp.tile([6, TV], F32, tag="rrd")
                        nc.sync.dma_start(rrd[:], rr[:])
                        for hp2 in range(hp - 2, hp + 1):
                            rb = rcpp.tile([P, TV], F32, tag="rb")
                            nc.sync.dma_start(
                                rb[0:64, :], _bcast_ap(rrd[2 * hp2 - g0], 64)
                            )
                            nc.sync.dma_start(
                                rb[64:128, :], _bcast_ap(rrd[2 * hp2 + 1 - g0], 64)
                            )
                            oc2 = oc_pair[hp2]
                            for po in (0, 64):
                                nc.vector.tensor_tensor(
                                    oc2[po : po + 64, :],
                                    oc2[po : po + 64, :],
                                    rb[po : po + 64, :],
                                    ALU.mult,
                                )
                            if dbg and b == 0:
                                nc.sync.dma_start(dbg_t[f"dbg_oc{hp2}"], oc2[:])

                prev_oc = oc_pair
                if b + 1 < BPC:
                    xT_b, v_win = emit_xv(b + 1)
                emit_proj(b, prev_oc)

    nc.finalize()
    return nc


_CACHE = {}


def _perm():
    """device col -> source token (0=CLS, 1+p=patch)."""
    perm = np.zeros(TV, np.int64)
    for w in range(NW):
        perm[97 * w : 97 * w + 96] = 1 + 96 * w + np.arange(96)
        perm[97 * w + 96] = 0
    return perm


def _make_in_maps(x, qkv_w, qkv_b, proj_w, proj_b):
    import ml_dtypes

    bf16 = ml_dtypes.bfloat16
    x = np.asarray(x, np.float32)
    qkv_w = np.asarray(qkv_w, np.float32)
    qkv_b = np.asarray(qkv_b, np.float32)
    proj_b = np.asarray(proj_b, np.float32)

    wqk = qkv_w[: 2 * C].copy()
    wqk[:C] *= SCALE                      # fold attention scale into Wq
    wqkT = np.ascontiguousarray(wqk.T).astype(bf16)
    wvT = np.ascontiguousarray(qkv_w[2 * C :].T).astype(bf16)
    wpT = np.ascontiguousarray(np.asarray(proj_w, np.float32).T).astype(bf16)
    bqk_h = qkv_b[: 2 * C].copy()
    bqk_h[:C] *= SCALE
    bv_h = np.ascontiguousarray(qkv_b[2 * C :])
    maskd = np.stack(_build_masks()).astype(bf16)
    ones_h = np.ones(H, bf16)
    perm = _perm()

    in_maps = []
    for c in range(NCORES):
        xc = x[c * BPC : (c + 1) * BPC]          # [BPC, N, C]
        xd = xc[:, perm, :]                      # [BPC, TV, C]
        xT_c = np.ascontiguousarray(xd.reshape(T, C).T).astype(bf16)
        in_maps.append(
            {
                "xT": xT_c,
                "wqkT": wqkT,
                "wvT": wvT,
                "wpT": wpT,
                "bqk": bqk_h,
                "bv": bv_h,
                "bp": proj_b,
                "maskd": maskd,
                "ones12": ones_h,
            }
        )
    return in_maps


def kernel(x, qkv_w, qkv_b, proj_w, proj_b):
    if "nc" not in _CACHE:
        _CACHE["nc"] = _build_program()
    nc = _CACHE["nc"]

    in_maps = _make_in_maps(x, qkv_w, qkv_b, proj_w, proj_b)
    res = run_bass_kernel_spmd(nc, in_maps, list(range(NCORES)))
    perm = _perm()
    inv = np.zeros(N, np.int64)
    inv[perm] = np.arange(TV)                    # src token -> device col (last copy wins for CLS)
    inv[0] = 96                                  # CLS from col 96 (summed copy)
    out = np.concatenate(
        [res.results[c]["y"].reshape(BPC, TV, C)[:, inv, :] for c in range(NCORES)],
        axis=0,
    )
    return np.ascontiguousarray(out.astype(np.float32))
